# revision 44
# baseline (speedup 1.0000x reference)
"""Distributed Trainium2 kernel for nn_CONNECT_86964497809993 (TGN-style
GNN message passing: last-event aggregation + GRU memory update + community
incidence matmul), sharded over 8 NeuronCores.

v4 strategy — all-e4m3 scaled weights + broadcast-paired residuals + fp8
community DoubleRow + half-bank psum slots:
  * Host does the index-only routing ('last' aggregation via stable-sort
    scatter) and COMPACTS to the ~63k nodes that received a message;
    untouched nodes copy through on the host and their community
    contribution is folded in on the host (pure-input preprocessing).
  * Weights ship as scaled hi/lo e4m3 pairs (hi = e4(W*64), lo =
    e4(W*64 - hi)); the GRU matmuls run fp8 DoubleRow, and the hh stacks
    pair (hi, lo) against a stride-0 broadcast of the single mem stream
    subtile, so the residual pass costs no extra weight-columns for the
    hh contraction. The 1/64 scale is folded into the sigmoid/tanh scale
    operand (free on ACT).
  * The gate psum is split into half-bank slots: rz pool = 3 banks/6
    slots (sigmoid reads 2-slot pairs), xn|hn pool = 4 banks/8 slots
    (DVE r*hn and xn+rhn run as 4-slot quads), community = 1 bank.
  * Elementwise runs quad-granular: ACT sigmoid pairs + tanh quads (with
    scale=1/64), DVE rhn/npre (psum-coupled) + d/zd (f16 SBUF 4x mode),
    Pool writes the final blend directly as fp8e4.
  * Community matmul is fp8e4 x fp8e4 DoubleRow over node pairs (out and
    incidence both e4m3), accumulating [M, C] in one psum bank.
  * Memory/incidence/out live in 4-node-interleaved DRAM rows (>=512B
    contiguous) so every DMA runs at full descriptor bandwidth; om ships
    fp8. The s/d/f/mem streams ship in ONE DMA per slab.
"""

import numpy as np
import ml_dtypes

from concourse import bacc
import concourse.mybir as mybir
from concourse.tile import TileContext
from concourse.bass_utils import run_bass_kernel_spmd

# Problem shapes (hardcoded per contract).
N, E, C = 100000, 50000, 256
M, D, F, T = 128, 128, 128, 64
NCORES = 8
P = 128

f32 = mybir.dt.float32
f16 = mybir.dt.float16
fp8 = mybir.dt.float8e4
A = mybir.AluOpType
AF = mybir.ActivationFunctionType
DR = mybir.MatmulPerfMode.DoubleRow

NP_FP8 = ml_dtypes.float8_e4m3

WSC = 64.0          # weight scale: hi/lo quantized at W*WSC, ACT rescales
WPK_B = 3968        # packed weight bytes per partition (incl identity)

_COMPILED = {}      # nt -> compiled Bacc program


def _slabs(nt):
    """Tile ranges per DMA slab. Quad-aligned (mult of 4) except the final
    2-tile tail. Small first slab for fast pipeline start, small last
    slabs for a short drain."""
    assert nt % 2 == 0
    main = nt - 2 if nt % 4 == 2 else nt
    cuts = [0]
    plan = [4, 8, 12, 12, 12, 8, 4] + [12] * 100
    for sz in plan:
        if cuts[-1] >= main:
            break
        cuts.append(min(main, cuts[-1] + sz))
    if main < nt:
        cuts.append(nt)
    return list(zip(cuts[:-1], cuts[1:]))


def _max_w(nt):
    return max(hi - lo for lo, hi in _slabs(nt)) * P


def _build_program(nt):
    assert nt % 2 == 0
    S = nt * P
    max_w = _max_w(nt)
    slabs = _slabs(nt)
    n_main = nt - 2 if nt % 4 == 2 else nt   # tiles in 4-interleave blocks
    nB = n_main // 4                          # 4-interleave dram blocks
    has_tail = n_main < nt

    nc = bacc.Bacc("TRN2", target_bir_lowering=False)

    # DRAM inputs
    sdm = nc.dram_tensor("sdm", [P, 4, S], fp8, kind="ExternalInput")
    ten = nc.dram_tensor("ten", [T, S], fp8, kind="ExternalInput")
    mem4 = nc.dram_tensor("mem4", [nB * P, 4 * M], f16, kind="ExternalInput")
    inc4 = nc.dram_tensor("inc4", [nB * P, 4 * C], fp8, kind="ExternalInput")
    wpk = nc.dram_tensor("wpk", [P, 2560], fp8, kind="ExternalInput")
    wpkn = nc.dram_tensor("wpkn", [P, WPK_B - 2560], fp8, kind="ExternalInput")
    cst = nc.dram_tensor("cst", [64, max_w], fp8, kind="ExternalInput")
    om4 = nc.dram_tensor("om4", [nB * P, 4 * M], fp8, kind="ExternalOutput")
    if has_tail:
        memt = nc.dram_tensor("memt", [P, 2 * M], f16, kind="ExternalInput")
        inct = nc.dram_tensor("inct", [P, 2 * C], fp8, kind="ExternalInput")
        omt = nc.dram_tensor("omt", [P, 2 * M], fp8, kind="ExternalOutput")
    ocm = nc.dram_tensor("ocm", [M, C], f32, kind="ExternalOutput")

    # quads: (t0, ntiles) — all 4 except a final 2-tile tail
    quads = [(q * 4, 4) for q in range(n_main // 4)]
    if has_tail:
        quads.append((n_main, 2))
    nq = len(quads)
    t2slab = {}
    for si, (lo, hi) in enumerate(slabs):
        for t in range(lo, hi):
            t2slab[t] = si

    with TileContext(nc) as tc:
        with tc.tile_pool(name="const", bufs=1) as cpool, \
             tc.tile_pool(name="work", bufs=3) as wpool, \
             tc.tile_pool(name="qwork", bufs=4) as qpool, \
             tc.tile_pool(name="psRZ", bufs=1, space="PSUM") as psRZ, \
             tc.tile_pool(name="psNH", bufs=1, space="PSUM") as psNH, \
             tc.tile_pool(name="psCM", bufs=1, space="PSUM") as psCM:

            wpk_t = cpool.tile([P, 2560], fp8)
            wpkn_t = cpool.tile([P, WPK_B - 2560], fp8)

            def wv(tile, lo, hi, b):
                return tile[:, lo:hi].rearrange("p (a b) -> p a b", b=b)

            w_sd_rz = wv(wpk_t, 0, 512, 256)
            w_ft_rz = wv(wpk_t, 512, 1024, 256)
            w_sd_rz_l = wv(wpk_t, 1024, 1536, 256)
            w_ft_rz_l = wv(wpk_t, 1536, 2048, 256)
            w_hh_rz = wv(wpk_t, 2048, 2560, 256)     # (hi, lo) pair
            w_sd_n = wv(wpkn_t, 0, 256, 128)
            w_ft_n = wv(wpkn_t, 256, 512, 128)
            w_sd_n_l = wv(wpkn_t, 512, 768, 128)
            w_ft_n_l = wv(wpkn_t, 768, 1024, 128)
            w_hh_n = wv(wpkn_t, 1024, 1280, 128)     # (hi, lo) pair
            w_id = wpkn_t[:, 1280:1408]              # fp8 identity

            # Stream stacks: subtiles 0..4 = s, d, f, mem, tenc(+bias lane).
            # cst fills partitions 64:128 of the tenc subtile once per buf,
            # emitted lazily on the buffer's first slab (startup ordering).
            sts = [cpool.tile([P, 5, max_w], fp8, tag=f"st{i}", name=f"st{i}")
                   for i in range(3)]

            # One tile per rotation slot (dep tracking is whole-tile):
            # rz pair-tiles rotate 3 deep (1 bank each), nh quad-tiles
            # rotate 2 deep (2 banks each), comm accumulator 1 bank.
            rzp = [psRZ.tile([P, 2, 2 * M], f32, tag=f"rz{k}", name=f"rz{k}")
                   for k in range(3)]
            nhp = [psNH.tile([P, 4, 2 * M], f32, tag=f"nh{k}", name=f"nh{k}")
                   for k in range(2)]
            comm = psCM.tile([M, C], f32)          # 1 bank

            slab_meta = {}
            state = {}

            def load_slab(si):
                lo, hi = slabs[si]
                w = (hi - lo) * P
                c0 = lo * P
                st = sts[si % 3]
                nc.sync.dma_start(st[:, 0:4, 0:w], sdm[:, :, c0:c0 + w])
                nc.sync.dma_start(st[0:T, 4, 0:w], ten[:, c0:c0 + w])
                if si == 0:
                    nc.sync.dma_start(wpk_t[:], wpk[:])
                    nc.sync.dma_start(wpkn_t[:], wpkn[:])
                if si < 3:
                    nc.gpsimd.dma_start(st[64:128, 4, :], cst[:])
                if lo < n_main:
                    nb = (hi - lo) // 4
                    r0 = (lo // 4) * P
                    mem_s = wpool.tile([P, nb, 4, M], f16, tag="mem")
                    nc.sync.dma_start(
                        mem_s[:], mem4[r0:r0 + nb * P].rearrange(
                            "(q p) (b f) -> p q b f", p=P, b=4))
                    inc_s = wpool.tile([P, nb, 4, C], fp8, tag="inc")
                    nc.gpsimd.dma_start(
                        inc_s[:], inc4[r0:r0 + nb * P].rearrange(
                            "(q p) (b f) -> p q b f", p=P, b=4))
                    out_s = wpool.tile([P, nb, 4, M], fp8, tag="out")
                    slab_meta[si] = dict(lo=lo, nb=nb, r0=r0, st=st,
                                         mem=mem_s, inc=inc_s, out=out_s)
                else:  # tail slab: 2 tiles, 2-interleave
                    mem_s = wpool.tile([P, 1, 2, M], f16, tag="memt")
                    nc.sync.dma_start(
                        mem_s[:],
                        memt[:].rearrange("p (b f) -> p b f", b=2).unsqueeze(1))
                    inc_s = wpool.tile([P, 1, 2, C], fp8, tag="inct")
                    nc.gpsimd.dma_start(
                        inc_s[:],
                        inct[:].rearrange("p (b f) -> p b f", b=2).unsqueeze(1))
                    out_s = wpool.tile([P, 1, 2, M], fp8, tag="outt")
                    slab_meta[si] = dict(lo=lo, nb=1, r0=None, st=st,
                                         mem=mem_s, inc=inc_s, out=out_s)

            def _tile_views(t):
                si = t2slab[t]
                if si not in slab_meta:
                    load_slab(si)
                st = slab_meta[si]["st"]
                cs = slice((t - slabs[si][0]) * P, (t - slabs[si][0] + 1) * P)
                sd = st[:, 0:2, cs]
                ft = st[:, 2:5:2, cs]                       # f & tenc subtiles
                mm = st[:, 3, cs].unsqueeze(1).broadcast_to([P, 2, P])
                return sd, ft, mm

            def stage_mm(i):
                # All rz MMs for the quad first, nh MMs last: the nh banks
                # (rotation 2) are released by npre(i-1) late in the previous
                # cycle — issuing their writers ~1.1us into this quad's PE
                # work keeps that chain off the critical path.
                # psum start/zero semantics are per 2KB bank: exactly ONE
                # start=True per bank; every other write first-touch-zeroes
                # its own bytes (the npre identity-like accumulate pattern
                # was reverted, but the one-start rule is kept).
                t0, g = quads[i]
                kw = dict(stop=False, perf_mode=DR, skip_group_check=True)
                for t in range(t0, t0 + g):
                    sd, ft, mm = _tile_views(t)
                    rz = rzp[(t // 2) % 3][:, t % 2, :]
                    nc.tensor.matmul(rz, sd, w_sd_rz, start=(t % 2 == 0), **kw)
                    nc.tensor.matmul(rz, ft, w_ft_rz, start=False, **kw)
                    nc.tensor.matmul(rz, sd, w_sd_rz_l, start=False, **kw)
                    nc.tensor.matmul(rz, ft, w_ft_rz_l, start=False, **kw)
                    nc.tensor.matmul(rz, mm, w_hh_rz, start=False,
                                     stop=(t % 2 == 1), perf_mode=DR,
                                     skip_group_check=True)
                for t in range(t0, t0 + g):
                    sd, ft, mm = _tile_views(t)
                    xn = nhp[(t // 4) % 2][:, t % 4, 0:M]
                    hn = nhp[(t // 4) % 2][:, t % 4, M:2 * M]
                    nc.tensor.matmul(xn, sd, w_sd_n,
                                     start=(t % 4 in (0, 2)), **kw)
                    nc.tensor.matmul(xn, ft, w_ft_n, start=False, **kw)
                    nc.tensor.matmul(xn, sd, w_sd_n_l, start=False, **kw)
                    nc.tensor.matmul(xn, ft, w_ft_n_l, start=False, **kw)
                    nc.tensor.matmul(hn, mm, w_hh_n, start=False,
                                     stop=(t % 4 in (1, 3)),
                                     perf_mode=DR, skip_group_check=True)

            def stage_sig(i):
                t0, g = quads[i]
                rz4 = qpool.tile([P, 4, 2 * M], f16, tag="rz4")
                for j in range(0, g, 2):
                    nc.scalar.activation(rz4[:, j:j + 2, :],
                                         rzp[((t0 + j) // 2) % 3][:],
                                         AF.Sigmoid, scale=1.0 / WSC)
                state[i] = dict(rz4=rz4)

            def stage_rhn(i):
                t0, g = quads[i]
                sv = state[i]
                nh = nhp[(t0 // 4) % 2]
                rhn4 = qpool.tile([P, 4, M], f16, tag="rhn4")
                nc.vector.tensor_tensor(rhn4[:, 0:g, :],
                                        sv["rz4"][:, 0:g, 0:M],
                                        nh[:, 0:g, M:2 * M], A.mult)
                npre4 = qpool.tile([P, 4, M], f16, tag="npre4")
                nc.vector.tensor_tensor(npre4[:, 0:g, :],
                                        nh[:, 0:g, 0:M],
                                        rhn4[:, 0:g, :], A.add)
                sv["npre4"] = npre4

            def stage_tanh(i):
                t0, g = quads[i]
                sv = state[i]
                n4 = qpool.tile([P, 4, M], f16, tag="n4")
                nc.scalar.activation(n4[:, 0:g, :], sv["npre4"][:, 0:g, :],
                                     AF.Tanh, scale=1.0 / WSC)
                sv["n4"] = n4

            def stage_dzd(i):
                t0, g = quads[i]
                sv = state[i]
                sm = slab_meta[t2slab[t0]]
                qb = (t0 - slabs[t2slab[t0]][0]) // 4
                d4 = qpool.tile([P, 4, M], f16, tag="d4")
                nc.vector.tensor_tensor(d4[:, 0:g, :],
                                        sm["mem"][:, qb, 0:g, :],
                                        sv["n4"][:, 0:g, :], A.subtract)
                zd4 = qpool.tile([P, 4, M], f16, tag="zd4")
                nc.vector.tensor_tensor(zd4[:, 0:g, :],
                                        sv["rz4"][:, 0:g, M:2 * M],
                                        d4[:, 0:g, :], A.mult)
                sv["zd4"] = zd4

            def stage_out(i):
                t0, g = quads[i]
                sv = state[i]
                si = t2slab[t0]
                sm = slab_meta[si]
                qb = (t0 - slabs[si][0]) // 4
                # last quads ride DVE: shorter op on the drain-critical path
                eng = nc.vector if i >= nq - 2 else nc.gpsimd
                eng.tensor_tensor(sm["out"][:, qb, 0:g, :],
                                  sv["n4"][:, 0:g, :],
                                  sv["zd4"][:, 0:g, :], A.add)

            def stage_comm(i):
                t0, g = quads[i]
                si = t2slab[t0]
                sm = slab_meta[si]
                qb = (t0 - slabs[si][0]) // 4
                for j in range(0, g, 2):
                    t = t0 + j
                    nc.tensor.matmul(comm[:], sm["out"][:, qb, j:j + 2, :],
                                     sm["inc"][:, qb, j:j + 2, :],
                                     start=(t == 0), stop=(t == nt - 2),
                                     perf_mode=DR)
                state.pop(i, None)
                # last quad of its slab -> flush om
                if i + 1 == nq or t2slab[quads[i + 1][0]] != si:
                    if sm["r0"] is not None:
                        nb = sm["nb"]
                        nc.sync.dma_start(
                            om4[sm["r0"]:sm["r0"] + nb * P].rearrange(
                                "(q p) (b f) -> p q b f", p=P, b=4),
                            sm["out"][:])
                    else:
                        nc.sync.dma_start(
                            omt[:].rearrange("p (b f) -> p b f", b=2)
                            .unsqueeze(1), sm["out"][:])

            # Emission order IS a valid serial order (bass2jax executes it
            # verbatim): all consumers of quad i-1's psum slots are emitted
            # before stage_mm(i) re-targets them.
            for i in range(nq + 3):
                if 1 <= i <= nq:
                    stage_sig(i - 1)
                    stage_rhn(i - 1)
                    stage_tanh(i - 1)
                if 2 <= i <= nq + 1:
                    stage_dzd(i - 2)
                    stage_out(i - 2)
                if 3 <= i <= nq + 2:
                    stage_comm(i - 3)
                if i < nq:
                    stage_mm(i)

            cm = qpool.tile([M, C], f32, tag="cm")
            nc.scalar.activation(cm[:], comm[:], AF.Copy)
            nc.sync.dma_start(ocm[:], cm[:])

    nc.compile()
    return nc


def _get_program(nt=62):
    if nt not in _COMPILED:
        _COMPILED[nt] = _build_program(nt)
    return _COMPILED[nt]


def _pack_weights(W_ih, W_hh, b_ih, b_hh):
    bias_row = (b_ih + b_hh).astype(np.float32)
    Wt_ext = np.zeros((P, 3 * M), np.float32)
    Wt_ext[0:T] = W_ih[2 * D + F:]
    Wt_ext[T] = bias_row
    Wih_full = np.concatenate([W_ih[0:2 * D + F], Wt_ext], axis=0)  # [512,384]
    Whh = W_hh.astype(np.float32)

    def hi_lo(Wf):
        hi = np.ascontiguousarray(Wf * WSC).astype(NP_FP8)
        lo = (Wf * WSC - hi.astype(np.float32)).astype(NP_FP8)
        return hi, lo

    ih_h, ih_l = hi_lo(Wih_full)
    hh_h, hh_l = hi_lo(Whh)

    def pair_rows(Wq, r0, r1, cols):
        # [128, 2, cols]: (row-block r0, row-block r1)
        return np.stack([Wq[r0:r0 + P, cols], Wq[r1:r1 + P, cols]], axis=1)

    rzc = slice(0, 2 * M)
    nnc = slice(2 * M, 3 * M)
    parts = [
        pair_rows(ih_h, 0, P, rzc), pair_rows(ih_h, 2 * P, 3 * P, rzc),
        pair_rows(ih_l, 0, P, rzc), pair_rows(ih_l, 2 * P, 3 * P, rzc),
        np.stack([hh_h[:, rzc], hh_l[:, rzc]], axis=1),
        pair_rows(ih_h, 0, P, nnc), pair_rows(ih_h, 2 * P, 3 * P, nnc),
        pair_rows(ih_l, 0, P, nnc), pair_rows(ih_l, 2 * P, 3 * P, nnc),
        np.stack([hh_h[:, nnc], hh_l[:, nnc]], axis=1),
        np.eye(P, dtype=np.float32).astype(NP_FP8),
    ]
    wpk_v = np.concatenate([p.reshape(P, -1).view(np.uint8) for p in parts],
                           axis=1).view(NP_FP8)
    assert wpk_v.shape == (P, WPK_B), wpk_v.shape
    return (np.ascontiguousarray(wpk_v[:, :2560]),
            np.ascontiguousarray(wpk_v[:, 2560:]))


def kernel(src, dst, t, last_update, event_feat, src_embeds, dst_embeds,
           nodes_memory, incidence, w_time, b_time, W_ih, W_hh, b_ih, b_hh):
    src = np.asarray(src); dst = np.asarray(dst); t = np.asarray(t)
    last_update = np.asarray(last_update)
    event_feat = np.asarray(event_feat, np.float32)
    src_embeds = np.asarray(src_embeds, np.float32)
    dst_embeds = np.asarray(dst_embeds, np.float32)
    nodes_memory = np.asarray(nodes_memory, np.float32)
    incidence = np.asarray(incidence, np.float32)
    w_time = np.asarray(w_time, np.float32); b_time = np.asarray(b_time, np.float32)
    W_ih = np.asarray(W_ih, np.float32); W_hh = np.asarray(W_hh, np.float32)
    b_ih = np.asarray(b_ih, np.float32); b_hh = np.asarray(b_hh, np.float32)

    # ---- Host routing: 'last' aggregation = stable-sort scatter (index-only)
    src_all = np.concatenate([src, dst])
    t_all = np.concatenate([t, t])
    perm = np.argsort(t_all, kind="stable")
    win = np.zeros(N, np.int64)
    win[src_all[perm]] = perm          # last write = newest event per node
    has = np.bincount(src_all, minlength=N) > 0
    nodes = np.nonzero(has)[0]         # compacted node ids (sorted)
    K = nodes.size

    # Per-core padded size (whole tiles; program handles 4k and 4k+2 tiles)
    Kc = -(-K // NCORES)
    nt = max(4, -(-Kc // P))
    if nt % 2:
        nt += 1
    S = nt * P
    n_main = nt - 2 if nt % 4 == 2 else nt
    nB = n_main // 4
    has_tail = n_main < nt
    nc_prog = _get_program(nt)

    # Winner event rows for the compacted nodes
    wn = win[nodes]
    lt = wn < E
    w0 = np.where(lt, wn, wn - E)
    emb_s = np.where(lt[:, None], src_embeds[w0], dst_embeds[w0])
    emb_d = np.where(lt[:, None], dst_embeds[w0], src_embeds[w0])
    feat = event_feat[w0]

    # Time encoding on host: fp32 arg (reference rounding), f64 cos, fp8 out
    dtw = (t_all[wn] - last_update[nodes]).astype(np.float32)
    x = dtw[:, None] * w_time[None, :] + b_time[None, :]
    tenc = np.cos(x.astype(np.float64)).astype(np.float32)

    memK = nodes_memory[nodes]
    incK = incidence[nodes]

    # Stream-order map: stream s = t*128 + p ->
    #   main: node 512*(t//4) + 4p + (t%4);  tail: 7680... + 2p + (t%2)
    ORD = np.empty(S, np.int64)
    tt, pp = np.meshgrid(np.arange(n_main), np.arange(P), indexing="ij")
    ORD[:n_main * P] = (512 * (tt // 4) + 4 * pp + tt % 4).reshape(-1)
    if has_tail:
        tt, pp = np.meshgrid(np.arange(2), np.arange(P), indexing="ij")
        ORD[n_main * P:] = (n_main * P + 2 * pp + tt).reshape(-1)

    wpk_v, wpkn_v = _pack_weights(W_ih, W_hh, b_ih, b_hh)
    cst_v = np.zeros((64, _max_w(nt)), NP_FP8)
    cst_v[0] = np.float32(1.0)        # bias lane (partition 64 of tenc subtile)

    in_maps = []
    core_n = []
    for c in range(NCORES):
        lo = c * Kc
        hi = min(K, (c + 1) * Kc)
        n = hi - lo
        core_n.append(n)
        so = ORD.copy()
        valid = so < n
        so = np.where(valid, so, 0)

        def stream(a):  # [n, 128] f32 -> [128, S] fp8 in stream order
            g = a[lo:lo + n][so] * valid[:, None]
            return np.ascontiguousarray(g.T).astype(NP_FP8)

        sdm_v = np.empty((P, 4, S), dtype=NP_FP8)
        sdm_v[:, 0] = stream(emb_s)
        sdm_v[:, 1] = stream(emb_d)
        sdm_v[:, 2] = stream(feat)
        sdm_v[:, 3] = stream(memK)
        ten_g = tenc[lo:lo + n][so] * valid[:, None]
        ten_v = np.ascontiguousarray(ten_g.T).astype(NP_FP8)

        def pad_rows(a, rows):
            out = np.zeros((rows,) + a.shape[1:], a.dtype)
            out[:a.shape[0]] = a
            return out

        mem_p = pad_rows(memK[lo:lo + n], S)
        inc_p = pad_rows(incK[lo:lo + n], S)
        # main 4-interleave rows: row 128*Q + p holds nodes 512Q + 4p + 0..3
        mem4_v = np.ascontiguousarray(
            mem_p[:n_main * P].reshape(nB, P, 4, M).reshape(nB * P, 4 * M)
        ).astype(np.float16)
        inc4_v = np.ascontiguousarray(
            inc_p[:n_main * P].reshape(nB, P, 4, C).reshape(nB * P, 4 * C)
        ).astype(NP_FP8)
        im = dict(sdm=sdm_v, ten=ten_v, mem4=mem4_v, inc4=inc4_v,
                  wpk=wpk_v, wpkn=wpkn_v, cst=cst_v)
        if has_tail:
            im["memt"] = np.ascontiguousarray(
                mem_p[n_main * P:].reshape(P, 2, M).reshape(P, 2 * M)
            ).astype(np.float16)
            im["inct"] = np.ascontiguousarray(
                inc_p[n_main * P:].reshape(P, 2, C).reshape(P, 2 * C)
            ).astype(NP_FP8)
        in_maps.append(im)

    res = run_bass_kernel_spmd(nc_prog, in_maps, core_ids=list(range(NCORES)))

    out = np.empty((N + C, M), np.float32)
    out[:N] = nodes_memory
    comm = np.zeros((M, C), np.float64)
    for c in range(NCORES):
        n = core_n[c]
        om_full = np.empty((S, M), np.float32)
        om_full[:n_main * P] = (res.results[c]["om4"]
                                .reshape(nB, P, 4, M)
                                .reshape(n_main * P, M).astype(np.float32))
        if has_tail:
            om_full[n_main * P:] = (res.results[c]["omt"]
                                    .reshape(P, 2, M)
                                    .reshape(2 * P, M).astype(np.float32))
        out[nodes[c * Kc:c * Kc + n]] = om_full[:n]
        comm += res.results[c]["ocm"]

    rest = incidence[~has].T.astype(np.float32) @ nodes_memory[~has]
    out[N:] = comm.T.astype(np.float32) + rest
    return out


# revision 45
# speedup vs baseline: 1.0047x; 1.0047x over previous
"""Distributed Trainium2 kernel for nn_CONNECT_86964497809993 (TGN-style
GNN message passing: last-event aggregation + GRU memory update + community
incidence matmul), sharded over 8 NeuronCores.

v4 strategy — all-e4m3 scaled weights + broadcast-paired residuals + fp8
community DoubleRow + half-bank psum slots:
  * Host does the index-only routing ('last' aggregation via stable-sort
    scatter) and COMPACTS to the ~63k nodes that received a message;
    untouched nodes copy through on the host and their community
    contribution is folded in on the host (pure-input preprocessing).
  * Weights ship as scaled hi/lo e4m3 pairs (hi = e4(W*64), lo =
    e4(W*64 - hi)); the GRU matmuls run fp8 DoubleRow, and the hh stacks
    pair (hi, lo) against a stride-0 broadcast of the single mem stream
    subtile, so the residual pass costs no extra weight-columns for the
    hh contraction. The 1/64 scale is folded into the sigmoid/tanh scale
    operand (free on ACT).
  * The gate psum is split into half-bank slots: rz pool = 3 banks/6
    slots (sigmoid reads 2-slot pairs), xn|hn pool = 4 banks/8 slots
    (DVE r*hn and xn+rhn run as 4-slot quads), community = 1 bank.
  * Elementwise runs quad-granular: ACT sigmoid pairs + tanh quads (with
    scale=1/64), DVE rhn/npre (psum-coupled) + d/zd (f16 SBUF 4x mode),
    Pool writes the final blend directly as fp8e4.
  * Community matmul is fp8e4 x fp8e4 DoubleRow over node pairs (out and
    incidence both e4m3), accumulating [M, C] in one psum bank.
  * Memory/incidence/out live in 4-node-interleaved DRAM rows (>=512B
    contiguous) so every DMA runs at full descriptor bandwidth; om ships
    fp8. The s/d/f/mem streams ship in ONE DMA per slab.
"""

import numpy as np
import ml_dtypes

from concourse import bacc
import concourse.mybir as mybir
from concourse.tile import TileContext
from concourse.bass_utils import run_bass_kernel_spmd

# Problem shapes (hardcoded per contract).
N, E, C = 100000, 50000, 256
M, D, F, T = 128, 128, 128, 64
NCORES = 8
P = 128

f32 = mybir.dt.float32
f16 = mybir.dt.float16
fp8 = mybir.dt.float8e4
A = mybir.AluOpType
AF = mybir.ActivationFunctionType
DR = mybir.MatmulPerfMode.DoubleRow

NP_FP8 = ml_dtypes.float8_e4m3

WSC = 64.0          # weight scale: hi/lo quantized at W*WSC, ACT rescales
WPK_B = 3968        # packed weight bytes per partition (incl identity)

_COMPILED = {}      # nt -> compiled Bacc program


def _slabs(nt):
    """Tile ranges per DMA slab. Quad-aligned (mult of 4) except the final
    2-tile tail. Small first slab for fast pipeline start, small last
    slabs for a short drain."""
    assert nt % 2 == 0
    main = nt - 2 if nt % 4 == 2 else nt
    cuts = [0]
    plan = [4, 8] + [12] * 100
    for sz in plan:
        if cuts[-1] >= main:
            break
        cuts.append(min(main, cuts[-1] + sz))
    if main < nt:
        cuts.append(nt)
    return list(zip(cuts[:-1], cuts[1:]))


def _max_w(nt):
    return max(hi - lo for lo, hi in _slabs(nt)) * P


def _build_program(nt):
    assert nt % 2 == 0
    S = nt * P
    max_w = _max_w(nt)
    slabs = _slabs(nt)
    n_main = nt - 2 if nt % 4 == 2 else nt   # tiles in 4-interleave blocks
    nB = n_main // 4                          # 4-interleave dram blocks
    has_tail = n_main < nt

    nc = bacc.Bacc("TRN2", target_bir_lowering=False)

    # DRAM inputs
    sdm = nc.dram_tensor("sdm", [P, 4, S], fp8, kind="ExternalInput")
    ten = nc.dram_tensor("ten", [T, S], fp8, kind="ExternalInput")
    mem4 = nc.dram_tensor("mem4", [nB * P, 4 * M], f16, kind="ExternalInput")
    inc4 = nc.dram_tensor("inc4", [nB * P, 4 * C], fp8, kind="ExternalInput")
    wpk = nc.dram_tensor("wpk", [P, 2560], fp8, kind="ExternalInput")
    wpkn = nc.dram_tensor("wpkn", [P, WPK_B - 2560], fp8, kind="ExternalInput")
    cst = nc.dram_tensor("cst", [64, max_w], fp8, kind="ExternalInput")
    om4 = nc.dram_tensor("om4", [nB * P, 4 * M], fp8, kind="ExternalOutput")
    if has_tail:
        memt = nc.dram_tensor("memt", [P, 2 * M], f16, kind="ExternalInput")
        inct = nc.dram_tensor("inct", [P, 2 * C], fp8, kind="ExternalInput")
        omt = nc.dram_tensor("omt", [P, 2 * M], fp8, kind="ExternalOutput")
    ocm = nc.dram_tensor("ocm", [M, C], f32, kind="ExternalOutput")

    # quads: (t0, ntiles) — all 4 except a final 2-tile tail
    quads = [(q * 4, 4) for q in range(n_main // 4)]
    if has_tail:
        quads.append((n_main, 2))
    nq = len(quads)
    t2slab = {}
    for si, (lo, hi) in enumerate(slabs):
        for t in range(lo, hi):
            t2slab[t] = si

    with TileContext(nc) as tc:
        with tc.tile_pool(name="const", bufs=1) as cpool, \
             tc.tile_pool(name="work", bufs=3) as wpool, \
             tc.tile_pool(name="qwork", bufs=4) as qpool, \
             tc.tile_pool(name="psRZ", bufs=1, space="PSUM") as psRZ, \
             tc.tile_pool(name="psNH", bufs=1, space="PSUM") as psNH, \
             tc.tile_pool(name="psCM", bufs=1, space="PSUM") as psCM:

            wpk_t = cpool.tile([P, 2560], fp8)
            wpkn_t = cpool.tile([P, WPK_B - 2560], fp8)

            def wv(tile, lo, hi, b):
                return tile[:, lo:hi].rearrange("p (a b) -> p a b", b=b)

            w_sd_rz = wv(wpk_t, 0, 512, 256)
            w_ft_rz = wv(wpk_t, 512, 1024, 256)
            w_sd_rz_l = wv(wpk_t, 1024, 1536, 256)
            w_ft_rz_l = wv(wpk_t, 1536, 2048, 256)
            w_hh_rz = wv(wpk_t, 2048, 2560, 256)     # (hi, lo) pair
            w_sd_n = wv(wpkn_t, 0, 256, 128)
            w_ft_n = wv(wpkn_t, 256, 512, 128)
            w_sd_n_l = wv(wpkn_t, 512, 768, 128)
            w_ft_n_l = wv(wpkn_t, 768, 1024, 128)
            w_hh_n = wv(wpkn_t, 1024, 1280, 128)     # (hi, lo) pair
            w_id = wpkn_t[:, 1280:1408]              # fp8 identity

            # Stream stacks: subtiles 0..4 = s, d, f, mem, tenc(+bias lane).
            # cst fills partitions 64:128 of the tenc subtile once per buf,
            # emitted lazily on the buffer's first slab (startup ordering).
            sts = [cpool.tile([P, 5, max_w], fp8, tag=f"st{i}", name=f"st{i}")
                   for i in range(3)]

            # One tile per rotation slot (dep tracking is whole-tile):
            # rz pair-tiles rotate 3 deep (1 bank each), nh quad-tiles
            # rotate 2 deep (2 banks each), comm accumulator 1 bank.
            rzp = [psRZ.tile([P, 2, 2 * M], f32, tag=f"rz{k}", name=f"rz{k}")
                   for k in range(3)]
            nhp = [psNH.tile([P, 4, 2 * M], f32, tag=f"nh{k}", name=f"nh{k}")
                   for k in range(2)]
            comm = psCM.tile([M, C], f32)          # 1 bank

            slab_meta = {}
            state = {}

            def load_slab(si):
                lo, hi = slabs[si]
                w = (hi - lo) * P
                c0 = lo * P
                st = sts[si % 3]
                nc.sync.dma_start(st[:, 0:4, 0:w], sdm[:, :, c0:c0 + w])
                nc.sync.dma_start(st[0:T, 4, 0:w], ten[:, c0:c0 + w])
                if si == 0:
                    nc.sync.dma_start(wpk_t[:], wpk[:])
                    nc.sync.dma_start(wpkn_t[:], wpkn[:])
                if si < 3:
                    nc.gpsimd.dma_start(st[64:128, 4, :], cst[:])
                if lo < n_main:
                    nb = (hi - lo) // 4
                    r0 = (lo // 4) * P
                    mem_s = wpool.tile([P, nb, 4, M], f16, tag="mem")
                    nc.sync.dma_start(
                        mem_s[:], mem4[r0:r0 + nb * P].rearrange(
                            "(q p) (b f) -> p q b f", p=P, b=4))
                    inc_s = wpool.tile([P, nb, 4, C], fp8, tag="inc")
                    nc.gpsimd.dma_start(
                        inc_s[:], inc4[r0:r0 + nb * P].rearrange(
                            "(q p) (b f) -> p q b f", p=P, b=4))
                    out_s = wpool.tile([P, nb, 4, M], fp8, tag="out")
                    slab_meta[si] = dict(lo=lo, nb=nb, r0=r0, st=st,
                                         mem=mem_s, inc=inc_s, out=out_s)
                else:  # tail slab: 2 tiles, 2-interleave
                    mem_s = wpool.tile([P, 1, 2, M], f16, tag="memt")
                    nc.sync.dma_start(
                        mem_s[:],
                        memt[:].rearrange("p (b f) -> p b f", b=2).unsqueeze(1))
                    inc_s = wpool.tile([P, 1, 2, C], fp8, tag="inct")
                    nc.gpsimd.dma_start(
                        inc_s[:],
                        inct[:].rearrange("p (b f) -> p b f", b=2).unsqueeze(1))
                    out_s = wpool.tile([P, 1, 2, M], fp8, tag="outt")
                    slab_meta[si] = dict(lo=lo, nb=1, r0=None, st=st,
                                         mem=mem_s, inc=inc_s, out=out_s)

            def _tile_views(t):
                si = t2slab[t]
                if si not in slab_meta:
                    load_slab(si)
                st = slab_meta[si]["st"]
                cs = slice((t - slabs[si][0]) * P, (t - slabs[si][0] + 1) * P)
                sd = st[:, 0:2, cs]
                ft = st[:, 2:5:2, cs]                       # f & tenc subtiles
                mm = st[:, 3, cs].unsqueeze(1).broadcast_to([P, 2, P])
                return sd, ft, mm

            def stage_mm(i):
                # All rz MMs for the quad first, nh MMs last: the nh banks
                # (rotation 2) are released by npre(i-1) late in the previous
                # cycle — issuing their writers ~1.1us into this quad's PE
                # work keeps that chain off the critical path.
                # psum start/zero semantics are per 2KB bank: exactly ONE
                # start=True per bank; every other write first-touch-zeroes
                # its own bytes (the npre identity-like accumulate pattern
                # was reverted, but the one-start rule is kept).
                t0, g = quads[i]
                kw = dict(stop=False, perf_mode=DR, skip_group_check=True)
                for t in range(t0, t0 + g):
                    sd, ft, mm = _tile_views(t)
                    rz = rzp[(t // 2) % 3][:, t % 2, :]
                    nc.tensor.matmul(rz, sd, w_sd_rz, start=(t % 2 == 0), **kw)
                    nc.tensor.matmul(rz, ft, w_ft_rz, start=False, **kw)
                    nc.tensor.matmul(rz, sd, w_sd_rz_l, start=False, **kw)
                    nc.tensor.matmul(rz, ft, w_ft_rz_l, start=False, **kw)
                    nc.tensor.matmul(rz, mm, w_hh_rz, start=False,
                                     stop=(t % 2 == 1), perf_mode=DR,
                                     skip_group_check=True)
                for t in range(t0, t0 + g):
                    sd, ft, mm = _tile_views(t)
                    xn = nhp[(t // 4) % 2][:, t % 4, 0:M]
                    hn = nhp[(t // 4) % 2][:, t % 4, M:2 * M]
                    nc.tensor.matmul(xn, sd, w_sd_n,
                                     start=(t % 4 in (0, 2)), **kw)
                    nc.tensor.matmul(xn, ft, w_ft_n, start=False, **kw)
                    nc.tensor.matmul(xn, sd, w_sd_n_l, start=False, **kw)
                    nc.tensor.matmul(xn, ft, w_ft_n_l, start=False, **kw)
                    nc.tensor.matmul(hn, mm, w_hh_n, start=False,
                                     stop=(t % 4 in (1, 3)),
                                     perf_mode=DR, skip_group_check=True)

            def stage_sig(i):
                t0, g = quads[i]
                rz4 = qpool.tile([P, 4, 2 * M], f16, tag="rz4")
                for j in range(0, g, 2):
                    nc.scalar.activation(rz4[:, j:j + 2, :],
                                         rzp[((t0 + j) // 2) % 3][:],
                                         AF.Sigmoid, scale=1.0 / WSC)
                state[i] = dict(rz4=rz4)

            def stage_rhn(i):
                t0, g = quads[i]
                sv = state[i]
                nh = nhp[(t0 // 4) % 2]
                rhn4 = qpool.tile([P, 4, M], f16, tag="rhn4")
                nc.vector.tensor_tensor(rhn4[:, 0:g, :],
                                        sv["rz4"][:, 0:g, 0:M],
                                        nh[:, 0:g, M:2 * M], A.mult)
                npre4 = qpool.tile([P, 4, M], f16, tag="npre4")
                nc.vector.tensor_tensor(npre4[:, 0:g, :],
                                        nh[:, 0:g, 0:M],
                                        rhn4[:, 0:g, :], A.add)
                sv["npre4"] = npre4

            def stage_tanh(i):
                t0, g = quads[i]
                sv = state[i]
                n4 = qpool.tile([P, 4, M], f16, tag="n4")
                nc.scalar.activation(n4[:, 0:g, :], sv["npre4"][:, 0:g, :],
                                     AF.Tanh, scale=1.0 / WSC)
                sv["n4"] = n4

            def stage_dzd(i):
                t0, g = quads[i]
                sv = state[i]
                sm = slab_meta[t2slab[t0]]
                qb = (t0 - slabs[t2slab[t0]][0]) // 4
                d4 = qpool.tile([P, 4, M], f16, tag="d4")
                nc.vector.tensor_tensor(d4[:, 0:g, :],
                                        sm["mem"][:, qb, 0:g, :],
                                        sv["n4"][:, 0:g, :], A.subtract)
                zd4 = qpool.tile([P, 4, M], f16, tag="zd4")
                nc.vector.tensor_tensor(zd4[:, 0:g, :],
                                        sv["rz4"][:, 0:g, M:2 * M],
                                        d4[:, 0:g, :], A.mult)
                sv["zd4"] = zd4

            def stage_out(i):
                t0, g = quads[i]
                sv = state[i]
                si = t2slab[t0]
                sm = slab_meta[si]
                qb = (t0 - slabs[si][0]) // 4
                # last quads ride DVE: shorter op on the drain-critical path
                eng = nc.vector if i >= nq - 2 else nc.gpsimd
                eng.tensor_tensor(sm["out"][:, qb, 0:g, :],
                                  sv["n4"][:, 0:g, :],
                                  sv["zd4"][:, 0:g, :], A.add)

            def stage_comm(i):
                t0, g = quads[i]
                si = t2slab[t0]
                sm = slab_meta[si]
                qb = (t0 - slabs[si][0]) // 4
                for j in range(0, g, 2):
                    t = t0 + j
                    nc.tensor.matmul(comm[:], sm["out"][:, qb, j:j + 2, :],
                                     sm["inc"][:, qb, j:j + 2, :],
                                     start=(t == 0), stop=(t == nt - 2),
                                     perf_mode=DR)
                state.pop(i, None)
                # last quad of its slab -> flush om
                if i + 1 == nq or t2slab[quads[i + 1][0]] != si:
                    if sm["r0"] is not None:
                        nb = sm["nb"]
                        nc.sync.dma_start(
                            om4[sm["r0"]:sm["r0"] + nb * P].rearrange(
                                "(q p) (b f) -> p q b f", p=P, b=4),
                            sm["out"][:])
                    else:
                        nc.sync.dma_start(
                            omt[:].rearrange("p (b f) -> p b f", b=2)
                            .unsqueeze(1), sm["out"][:])

            # Emission order IS a valid serial order (bass2jax executes it
            # verbatim): all consumers of quad i-1's psum slots are emitted
            # before stage_mm(i) re-targets them.
            for i in range(nq + 3):
                if 1 <= i <= nq:
                    stage_sig(i - 1)
                    stage_rhn(i - 1)
                    stage_tanh(i - 1)
                if 2 <= i <= nq + 1:
                    stage_dzd(i - 2)
                    stage_out(i - 2)
                if 3 <= i <= nq + 2:
                    stage_comm(i - 3)
                if i < nq:
                    stage_mm(i)

            cm = qpool.tile([M, C], f32, tag="cm")
            nc.scalar.activation(cm[:], comm[:], AF.Copy)
            nc.sync.dma_start(ocm[:], cm[:])

    nc.compile()
    return nc


def _get_program(nt=62):
    if nt not in _COMPILED:
        _COMPILED[nt] = _build_program(nt)
    return _COMPILED[nt]


def _pack_weights(W_ih, W_hh, b_ih, b_hh):
    bias_row = (b_ih + b_hh).astype(np.float32)
    Wt_ext = np.zeros((P, 3 * M), np.float32)
    Wt_ext[0:T] = W_ih[2 * D + F:]
    Wt_ext[T] = bias_row
    Wih_full = np.concatenate([W_ih[0:2 * D + F], Wt_ext], axis=0)  # [512,384]
    Whh = W_hh.astype(np.float32)

    def hi_lo(Wf):
        hi = np.ascontiguousarray(Wf * WSC).astype(NP_FP8)
        lo = (Wf * WSC - hi.astype(np.float32)).astype(NP_FP8)
        return hi, lo

    ih_h, ih_l = hi_lo(Wih_full)
    hh_h, hh_l = hi_lo(Whh)

    def pair_rows(Wq, r0, r1, cols):
        # [128, 2, cols]: (row-block r0, row-block r1)
        return np.stack([Wq[r0:r0 + P, cols], Wq[r1:r1 + P, cols]], axis=1)

    rzc = slice(0, 2 * M)
    nnc = slice(2 * M, 3 * M)
    parts = [
        pair_rows(ih_h, 0, P, rzc), pair_rows(ih_h, 2 * P, 3 * P, rzc),
        pair_rows(ih_l, 0, P, rzc), pair_rows(ih_l, 2 * P, 3 * P, rzc),
        np.stack([hh_h[:, rzc], hh_l[:, rzc]], axis=1),
        pair_rows(ih_h, 0, P, nnc), pair_rows(ih_h, 2 * P, 3 * P, nnc),
        pair_rows(ih_l, 0, P, nnc), pair_rows(ih_l, 2 * P, 3 * P, nnc),
        np.stack([hh_h[:, nnc], hh_l[:, nnc]], axis=1),
        np.eye(P, dtype=np.float32).astype(NP_FP8),
    ]
    wpk_v = np.concatenate([p.reshape(P, -1).view(np.uint8) for p in parts],
                           axis=1).view(NP_FP8)
    assert wpk_v.shape == (P, WPK_B), wpk_v.shape
    return (np.ascontiguousarray(wpk_v[:, :2560]),
            np.ascontiguousarray(wpk_v[:, 2560:]))


def kernel(src, dst, t, last_update, event_feat, src_embeds, dst_embeds,
           nodes_memory, incidence, w_time, b_time, W_ih, W_hh, b_ih, b_hh):
    src = np.asarray(src); dst = np.asarray(dst); t = np.asarray(t)
    last_update = np.asarray(last_update)
    event_feat = np.asarray(event_feat, np.float32)
    src_embeds = np.asarray(src_embeds, np.float32)
    dst_embeds = np.asarray(dst_embeds, np.float32)
    nodes_memory = np.asarray(nodes_memory, np.float32)
    incidence = np.asarray(incidence, np.float32)
    w_time = np.asarray(w_time, np.float32); b_time = np.asarray(b_time, np.float32)
    W_ih = np.asarray(W_ih, np.float32); W_hh = np.asarray(W_hh, np.float32)
    b_ih = np.asarray(b_ih, np.float32); b_hh = np.asarray(b_hh, np.float32)

    # ---- Host routing: 'last' aggregation = stable-sort scatter (index-only)
    src_all = np.concatenate([src, dst])
    t_all = np.concatenate([t, t])
    perm = np.argsort(t_all, kind="stable")
    win = np.zeros(N, np.int64)
    win[src_all[perm]] = perm          # last write = newest event per node
    has = np.bincount(src_all, minlength=N) > 0
    nodes = np.nonzero(has)[0]         # compacted node ids (sorted)
    K = nodes.size

    # Per-core padded size (whole tiles; program handles 4k and 4k+2 tiles)
    Kc = -(-K // NCORES)
    nt = max(4, -(-Kc // P))
    if nt % 2:
        nt += 1
    S = nt * P
    n_main = nt - 2 if nt % 4 == 2 else nt
    nB = n_main // 4
    has_tail = n_main < nt
    nc_prog = _get_program(nt)

    # Winner event rows for the compacted nodes
    wn = win[nodes]
    lt = wn < E
    w0 = np.where(lt, wn, wn - E)
    emb_s = np.where(lt[:, None], src_embeds[w0], dst_embeds[w0])
    emb_d = np.where(lt[:, None], dst_embeds[w0], src_embeds[w0])
    feat = event_feat[w0]

    # Time encoding on host: fp32 arg (reference rounding), f64 cos, fp8 out
    dtw = (t_all[wn] - last_update[nodes]).astype(np.float32)
    x = dtw[:, None] * w_time[None, :] + b_time[None, :]
    tenc = np.cos(x.astype(np.float64)).astype(np.float32)

    memK = nodes_memory[nodes]
    incK = incidence[nodes]

    # Stream-order map: stream s = t*128 + p ->
    #   main: node 512*(t//4) + 4p + (t%4);  tail: 7680... + 2p + (t%2)
    ORD = np.empty(S, np.int64)
    tt, pp = np.meshgrid(np.arange(n_main), np.arange(P), indexing="ij")
    ORD[:n_main * P] = (512 * (tt // 4) + 4 * pp + tt % 4).reshape(-1)
    if has_tail:
        tt, pp = np.meshgrid(np.arange(2), np.arange(P), indexing="ij")
        ORD[n_main * P:] = (n_main * P + 2 * pp + tt).reshape(-1)

    wpk_v, wpkn_v = _pack_weights(W_ih, W_hh, b_ih, b_hh)
    cst_v = np.zeros((64, _max_w(nt)), NP_FP8)
    cst_v[0] = np.float32(1.0)        # bias lane (partition 64 of tenc subtile)

    in_maps = []
    core_n = []
    for c in range(NCORES):
        lo = c * Kc
        hi = min(K, (c + 1) * Kc)
        n = hi - lo
        core_n.append(n)
        so = ORD.copy()
        valid = so < n
        so = np.where(valid, so, 0)

        def stream(a):  # [n, 128] f32 -> [128, S] fp8 in stream order
            g = a[lo:lo + n][so] * valid[:, None]
            return np.ascontiguousarray(g.T).astype(NP_FP8)

        sdm_v = np.empty((P, 4, S), dtype=NP_FP8)
        sdm_v[:, 0] = stream(emb_s)
        sdm_v[:, 1] = stream(emb_d)
        sdm_v[:, 2] = stream(feat)
        sdm_v[:, 3] = stream(memK)
        ten_g = tenc[lo:lo + n][so] * valid[:, None]
        ten_v = np.ascontiguousarray(ten_g.T).astype(NP_FP8)

        def pad_rows(a, rows):
            out = np.zeros((rows,) + a.shape[1:], a.dtype)
            out[:a.shape[0]] = a
            return out

        mem_p = pad_rows(memK[lo:lo + n], S)
        inc_p = pad_rows(incK[lo:lo + n], S)
        # main 4-interleave rows: row 128*Q + p holds nodes 512Q + 4p + 0..3
        mem4_v = np.ascontiguousarray(
            mem_p[:n_main * P].reshape(nB, P, 4, M).reshape(nB * P, 4 * M)
        ).astype(np.float16)
        inc4_v = np.ascontiguousarray(
            inc_p[:n_main * P].reshape(nB, P, 4, C).reshape(nB * P, 4 * C)
        ).astype(NP_FP8)
        im = dict(sdm=sdm_v, ten=ten_v, mem4=mem4_v, inc4=inc4_v,
                  wpk=wpk_v, wpkn=wpkn_v, cst=cst_v)
        if has_tail:
            im["memt"] = np.ascontiguousarray(
                mem_p[n_main * P:].reshape(P, 2, M).reshape(P, 2 * M)
            ).astype(np.float16)
            im["inct"] = np.ascontiguousarray(
                inc_p[n_main * P:].reshape(P, 2, C).reshape(P, 2 * C)
            ).astype(NP_FP8)
        in_maps.append(im)

    res = run_bass_kernel_spmd(nc_prog, in_maps, core_ids=list(range(NCORES)))

    out = np.empty((N + C, M), np.float32)
    out[:N] = nodes_memory
    comm = np.zeros((M, C), np.float64)
    for c in range(NCORES):
        n = core_n[c]
        om_full = np.empty((S, M), np.float32)
        om_full[:n_main * P] = (res.results[c]["om4"]
                                .reshape(nB, P, 4, M)
                                .reshape(n_main * P, M).astype(np.float32))
        if has_tail:
            om_full[n_main * P:] = (res.results[c]["omt"]
                                    .reshape(P, 2, M)
                                    .reshape(2 * P, M).astype(np.float32))
        out[nodes[c * Kc:c * Kc + n]] = om_full[:n]
        comm += res.results[c]["ocm"]

    rest = incidence[~has].T.astype(np.float32) @ nodes_memory[~has]
    out[N:] = comm.T.astype(np.float32) + rest
    return out
